# revision 20
# baseline (speedup 1.0000x reference)
"""Trainium2 Bass kernel for nn_ConditionalDisCoLoss.

loss = BCEWithLogits(inputs, targets)
     + dCor_masked(sigmoid(inputs), spectators, mask=spectators>=0.5)

Host/device split (the sharding hint's "small filtered 1-D vectors"):
  * Host filters samples by the mask (c ~= 4080 of 8192 survive) and pads
    to CPAD; the dCor pair matrices shrink from n^2 to c^2 (~4x less work).
  * Host computes every O(c log c) term exactly in float64: row sums
    A_i = sum_j |p_i-p_j| and B_i (sort + prefix sums), Sxx/Syy closed
    forms, Tx/Ty/sAB/sAA/sBB.
  * Device computes the only quadratic term Sxy = sum_ij m_i m_j
    |p_i-p_j||s_i-s_j| plus the (linear) BCE partial sums.

Device per tile [128 x 1024] (jt >= it bands, round-robin i-tiles so all
8 cores run identical 10-tile programs):
  PE : masked pairwise diffs D1 = m_i m_j (p_i-p_j), D2 likewise for s,
       via rank-2 f32r matmuls (2 PSUM banks each, 4 matmuls)
  ACT: U = |D1|  (f32, PSUM->SBUF)
  DVE: P = U * D2  (bf16 out; |U*D2| == |D1||D2|; only DVE can read PSUM)
  DVE+Pool: |P| with fused row-sum accum -> Sxy partial columns; the DVE
       slice runs in 4x perf mode (all-bf16 SBUF operands), the idle
       gpsimd engine absorbs the wide remainder
BCE runs at the end on ACT (softplus, same act table as Abs) + DVE.
Host sums the [128, ncol] partials in float64 and assembles the loss.
"""

import numpy as np
from contextlib import ExitStack

import concourse.bass as bass
import concourse.bacc as bacc
import concourse.tile as tile
from concourse import mybir
from concourse.bass_utils import run_bass_kernel_spmd

N = 8192
NCORES = 8
P = 128
JT = 1024
CPAD = 4096              # padded filtered size (c=4080 for the reference seed)
CPAD_BIG = 5120          # fallback variant if c > 4096
WD = 480                 # columns of the |P| pass done on ACT; rest on DVE

F32 = mybir.dt.float32
BF16 = mybir.dt.bfloat16
F32R = mybir.dt.float32r
ALU = mybir.AluOpType
ACTF = mybir.ActivationFunctionType
AX = mybir.AxisListType

BSH = N // NCORES        # 1024 BCE samples per core
BCOL = BSH // P          # 8


def _tile_weights(nb):
    """Per-tile Sxy weights: diagonal-band tile once, strictly-upper twice."""
    w = []
    for it in range(nb):
        for jj in range(nb - it):
            w.append(1.0 if jj == 0 else 2.0)
    return w


def _build(cpad):
    nb = cpad // JT          # bands == i-tiles per core
    rows = cpad // NCORES    # stationary rows per core
    ntiles = nb * (nb + 1) // 2
    ncol = 2 * ntiles + 3    # 2 Sxy cols per tile + relu/ln1pexp/x*t sums

    nc = bacc.Bacc("TRN2", target_bir_lowering=False, debug=False,
                   num_devices=NCORES, enable_asserts=False)

    ra = nc.dram_tensor("ra", [2, cpad], F32R, kind="ExternalInput")
    rb = nc.dram_tensor("rb", [2, cpad], F32R, kind="ExternalInput")
    la = nc.dram_tensor("la", [2, rows], F32R, kind="ExternalInput")
    lb = nc.dram_tensor("lb", [2, rows], F32R, kind="ExternalInput")
    xs = nc.dram_tensor("xs", [P, BCOL], F32, kind="ExternalInput")
    ts = nc.dram_tensor("ts", [P, BCOL], F32, kind="ExternalInput")
    out = nc.dram_tensor("acc", [P, ncol], F32, kind="ExternalOutput")

    with tile.TileContext(nc) as tc, ExitStack() as ctx:
        pre = ctx.enter_context(tc.tile_pool(name="pre", bufs=1))
        uvp = ctx.enter_context(tc.tile_pool(name="uv", bufs=4))
        psp = ctx.enter_context(tc.tile_pool(name="psp", bufs=2, space="PSUM"))

        rat = pre.tile([2, cpad], F32R)
        rbt = pre.tile([2, cpad], F32R)
        lat = pre.tile([2, rows], F32R)
        lbt = pre.tile([2, rows], F32R)
        xst = pre.tile([P, BCOL], F32)
        tst = pre.tile([P, BCOL], F32)
        nc.sync.dma_start(out=rat, in_=ra.ap())
        nc.scalar.dma_start(out=rbt, in_=rb.ap())
        nc.sync.dma_start(out=lat, in_=la.ap())
        nc.scalar.dma_start(out=lbt, in_=lb.ap())
        nc.sync.dma_start(out=xst, in_=xs.ap())
        nc.scalar.dma_start(out=tst, in_=ts.ap())

        racc = pre.tile([P, ncol], F32)

        # BCE partials: softplus(x) = relu(x) + ln(1+exp(-|x|)), so
        # bce = (relu_sum + ln_sum - xt_sum)/N. Abs/Relu are in every act
        # table; Exp+Ln share natural_log_exp_and_others -> one reload max.
        rxj = pre.tile([P, BCOL], F32)
        nc.scalar.activation(rxj, xst, ACTF.Relu,
                             accum_out=racc[:, 2 * ntiles:2 * ntiles + 1])
        axj = pre.tile([P, BCOL], F32)
        nc.scalar.activation(axj, xst, ACTF.Abs)
        enj = pre.tile([P, BCOL], F32)
        nc.scalar.activation(enj, axj, ACTF.Exp, scale=-1.0)
        lnj = pre.tile([P, BCOL], F32)
        nc.scalar.activation(lnj, enj, ACTF.Ln, bias=1.0,
                             accum_out=racc[:, 2 * ntiles + 1:2 * ntiles + 2])
        xtj = pre.tile([P, BCOL], F32)
        nc.vector.scalar_tensor_tensor(out=xtj, in0=xst, scalar=0.0,
                                       in1=tst, op0=ALU.bypass, op1=ALU.mult,
                                       accum_out=racc[:, 2 * ntiles + 2:2 * ntiles + 3])

        tix = 0
        for it in range(nb):
            lA = lat[:, it * P:(it + 1) * P]
            lB = lbt[:, it * P:(it + 1) * P]
            for jj in range(nb - it):
                jt = it + jj
                psA = psp.tile([P, JT], F32, tag="psA")
                psB = psp.tile([P, JT], F32, tag="psB")
                for h in range(JT // 512):
                    j0 = jt * JT + h * 512
                    nc.tensor.matmul(psA[:, h * 512:(h + 1) * 512],
                                     lhsT=lA, rhs=rat[:, j0:j0 + 512],
                                     start=True, stop=True)
                    nc.tensor.matmul(psB[:, h * 512:(h + 1) * 512],
                                     lhsT=lB, rhs=rbt[:, j0:j0 + 512],
                                     start=True, stop=True)
                U = uvp.tile([P, JT], F32, tag="U")
                nc.scalar.activation(U, psA, ACTF.Abs)
                # Signed product P = |D1| * D2 (only DVE can read PSUM), then
                # |P| = max(-P, P) in one stt per slice with fused row-sum
                # accum (Sxy partials). P is SBUF, so the idle gpsimd engine
                # absorbs the wide remainder of the abs pass.
                Pt = uvp.tile([P, JT], F32, tag="P")
                nc.vector.scalar_tensor_tensor(out=Pt, in0=U, scalar=0.0,
                                               in1=psB,
                                               op0=ALU.bypass, op1=ALU.mult)
                # |P| with fused row-sum accum, split ACT (activation Abs,
                # which idles otherwise) / DVE (stt max(-P, P)).
                Wt = uvp.tile([P, JT], F32, tag="W")
                nc.scalar.activation(Wt[:, 0:WD], Pt[:, 0:WD], ACTF.Abs,
                                     accum_out=racc[:, 2 * tix:2 * tix + 1])
                nc.vector.scalar_tensor_tensor(out=Wt[:, WD:JT],
                                               in0=Pt[:, WD:JT], scalar=-1.0,
                                               in1=Pt[:, WD:JT],
                                               op0=ALU.mult, op1=ALU.max,
                                               accum_out=racc[:, 2 * tix + 1:2 * tix + 2])
                tix += 1

        nc.sync.dma_start(out=out.ap(), in_=racc)

    nc.compile()
    return nc


_NC_CACHE = {}


def _get_nc(cpad):
    if cpad not in _NC_CACHE:
        _NC_CACHE[cpad] = _build(cpad)
    return _NC_CACHE[cpad]


def _row_index(k, cpad):
    """Filtered-space row indices owned by core k (i-tiles NCORES*t + k)."""
    nit = cpad // JT
    idx = []
    for t in range(nit):
        g = NCORES * t + k
        idx.append(np.arange(g * P, (g + 1) * P))
    return np.concatenate(idx)


def _rowsums_abs(v):
    """A_i = sum_j |v_i - v_j| in O(c log c), exact float64."""
    o = np.argsort(v, kind="stable")
    q = v[o]
    pre = np.cumsum(q)
    tot = pre[-1]
    k = np.arange(len(q), dtype=np.float64)
    s = q * (2.0 * k + 2.0 - len(q)) - 2.0 * pre + tot
    a = np.empty_like(v)
    a[o] = s
    return a


def _numpy_loss(x, t, s):
    """Full-precision fallback (c > CPAD_BIG or degenerate inputs)."""
    x64 = x.astype(np.float64).reshape(-1)
    t64 = t.astype(np.float64).reshape(-1)
    s64 = s.astype(np.float64).reshape(-1)
    bce = np.mean(np.maximum(x64, 0) - x64 * t64 + np.log1p(np.exp(-np.abs(x64))))
    m = s.reshape(-1) >= np.float32(0.5)
    c = int(m.sum())
    if c == 0:
        return np.float32(bce)
    p = (1.0 / (1.0 + np.exp(-x64))).astype(np.float32)[m].astype(np.float64)
    ss = s.reshape(-1)[m].astype(np.float64)
    dx = np.abs(p[:, None] - p[None, :])
    dy = np.abs(ss[:, None] - ss[None, :])
    Sxy = float((dx * dy).sum())
    A = dx.sum(1); B = dy.sum(1)
    loss = bce + _assemble_dcor(c, Sxy, A, B, p, ss)
    return np.float32(loss)


def _assemble_dcor(c, Sxy, A, B, p, ss):
    Sxx = 2.0 * c * float((p * p).sum()) - 2.0 * float(p.sum()) ** 2
    Syy = 2.0 * c * float((ss * ss).sum()) - 2.0 * float(ss.sum()) ** 2
    Tx, Ty = float(A.sum()), float(B.sum())
    sAB, sAA, sBB = float(A @ B), float(A @ A), float(B @ B)
    Vxy = Sxy - 2.0 / c * sAB + Tx * Ty / c ** 2
    Vxx = Sxx - 2.0 / c * sAA + Tx * Tx / c ** 2
    Vyy = Syy - 2.0 / c * sBB + Ty * Ty / c ** 2
    EPS = 1e-8
    dcov = np.sqrt(max(Vxy / c ** 2, EPS))
    dvx = np.sqrt(max(Vxx / c ** 2, EPS))
    dvy = np.sqrt(max(Vyy / c ** 2, EPS))
    return dcov / (dvx * dvy)


def _prepare(inputs, targets, spectators):
    x = np.ascontiguousarray(np.asarray(inputs, dtype=np.float32)).reshape(N)
    t = np.ascontiguousarray(np.asarray(targets, dtype=np.float32)).reshape(N)
    s = np.ascontiguousarray(np.asarray(spectators, dtype=np.float32)).reshape(N)

    m = s >= np.float32(0.5)
    c = int(m.sum())
    cpad = CPAD if c <= CPAD else (CPAD_BIG if c <= CPAD_BIG else None)
    if cpad is None or c == 0:
        return None, None, None, (x, t, s)

    # p in f32, used consistently by host (A, Sxx) and device (Sxy)
    p32 = (1.0 / (1.0 + np.exp(-x.astype(np.float64)))).astype(np.float32)
    p_sel = p32[m]
    s_sel = s[m]

    p_pad = np.zeros(cpad, np.float32); p_pad[:c] = p_sel
    s_pad = np.zeros(cpad, np.float32); s_pad[:c] = s_sel
    m_pad = np.zeros(cpad, np.float32); m_pad[:c] = 1.0

    ra = np.ascontiguousarray(np.stack([m_pad, p_pad]))
    rb = np.ascontiguousarray(np.stack([m_pad, s_pad]))

    in_maps = []
    for k in range(NCORES):
        idx = _row_index(k, cpad)
        la = np.ascontiguousarray(np.stack([p_pad[idx], -m_pad[idx]]))
        lb = np.ascontiguousarray(np.stack([s_pad[idx], -m_pad[idx]]))
        xsk = np.ascontiguousarray(x[k * BSH:(k + 1) * BSH].reshape(P, BCOL))
        tsk = np.ascontiguousarray(t[k * BSH:(k + 1) * BSH].reshape(P, BCOL))
        in_maps.append({"ra": ra, "rb": rb, "la": la, "lb": lb,
                        "xs": xsk, "ts": tsk})

    meta = {
        "c": c, "cpad": cpad,
        "p_sel": p_sel.astype(np.float64),
        "s_sel": s_sel.astype(np.float64),
    }
    return cpad, in_maps, meta, None


def _combine(results, meta):
    cpad = meta["cpad"]
    nb = cpad // JT
    ntiles = nb * (nb + 1) // 2
    w = np.array(_tile_weights(nb), np.float64)

    Sxy = 0.0
    sp_sum = 0.0
    xt_sum = 0.0
    for res in results:
        cols = res["acc"].astype(np.float64).sum(axis=0)
        rt = cols[:2 * ntiles].reshape(ntiles, 2).sum(axis=1)
        Sxy += float(rt @ w)
        sp_sum += float(cols[2 * ntiles]) + float(cols[2 * ntiles + 1])
        xt_sum += float(cols[2 * ntiles + 2])

    bce = (sp_sum - xt_sum) / N
    c = meta["c"]
    p = meta["p_sel"]; ss = meta["s_sel"]
    A = _rowsums_abs(p); B = _rowsums_abs(ss)
    loss = bce + _assemble_dcor(c, Sxy, A, B, p, ss)
    return np.float32(loss)


def kernel(inputs, targets, spectators):
    cpad, in_maps, meta, fb = _prepare(inputs, targets, spectators)
    if fb is not None:
        return _numpy_loss(*fb)
    nc = _get_nc(cpad)
    res = run_bass_kernel_spmd(nc, in_maps, list(range(NCORES)))
    return _combine(res.results, meta)


if __name__ == "__main__":
    d = np.load("/root/problem/cached_io.npz")
    out = kernel(d["inputs"], d["targets"], d["spectators"])
    exp = float(d["expected"])
    rel = abs(float(out) - exp) / abs(exp)
    print(f"kernel: {float(out):.8f}  expected: {exp:.8f}  rel err: {rel:.3e}")


# revision 21
# speedup vs baseline: 1.0254x; 1.0254x over previous
"""Trainium2 Bass kernel for nn_ConditionalDisCoLoss.

loss = BCEWithLogits(inputs, targets)
     + dCor_masked(sigmoid(inputs), spectators, mask=spectators>=0.5)

Host/device split (the sharding hint's "small filtered 1-D vectors"):
  * Host filters samples by the mask (c ~= 4080 of 8192 survive) and pads
    to CPAD; the dCor pair matrices shrink from n^2 to c^2 (~4x less work).
  * Host computes every O(c log c) term exactly in float64: row sums
    A_i = sum_j |p_i-p_j| and B_i (sort + prefix sums), Sxx/Syy closed
    forms, Tx/Ty/sAB/sAA/sBB.
  * Device computes the only quadratic term Sxy = sum_ij m_i m_j
    |p_i-p_j||s_i-s_j| plus the (linear) BCE partial sums.

Device per tile [128 x 1024] (jt >= it bands, round-robin i-tiles so all
8 cores run identical 10-tile programs):
  PE : masked pairwise diffs D1 = m_i m_j (p_i-p_j), D2 likewise for s,
       via rank-2 f32r matmuls (2 PSUM banks each, 4 matmuls)
  ACT: U = |D1|  (f32, PSUM->SBUF)
  DVE: P = U * D2  (bf16 out; |U*D2| == |D1||D2|; only DVE can read PSUM)
  DVE+Pool: |P| with fused row-sum accum -> Sxy partial columns; the DVE
       slice runs in 4x perf mode (all-bf16 SBUF operands), the idle
       gpsimd engine absorbs the wide remainder
BCE runs at the end on ACT (softplus, same act table as Abs) + DVE.
Host sums the [128, ncol] partials in float64 and assembles the loss.
"""

import numpy as np
from contextlib import ExitStack

import concourse.bass as bass
import concourse.bacc as bacc
import concourse.tile as tile
from concourse import mybir
from concourse.bass_utils import run_bass_kernel_spmd

N = 8192
NCORES = 8
P = 128
JT = 1024
CPAD = 4096              # padded filtered size (c=4080 for the reference seed)
CPAD_BIG = 5120          # fallback variant if c > 4096
WD = 480                 # columns of the |P| pass done on ACT; rest on DVE

F32 = mybir.dt.float32
BF16 = mybir.dt.bfloat16
F32R = mybir.dt.float32r
ALU = mybir.AluOpType
ACTF = mybir.ActivationFunctionType
AX = mybir.AxisListType

BSH = N // NCORES        # 1024 BCE samples per core
BCOL = BSH // P          # 8


def _tile_weights(nb):
    """Per-tile Sxy weights: diagonal-band tile once, strictly-upper twice."""
    w = []
    for it in range(nb):
        for jj in range(nb - it):
            w.append(1.0 if jj == 0 else 2.0)
    return w


def _build(cpad):
    nb = cpad // JT          # bands == i-tiles per core
    rows = cpad // NCORES    # stationary rows per core
    ntiles = nb * (nb + 1) // 2
    ncol = 2 * ntiles + 3    # 2 Sxy cols per tile + relu/ln1pexp/x*t sums

    nc = bacc.Bacc("TRN2", target_bir_lowering=False, debug=False,
                   num_devices=NCORES, enable_asserts=False)

    ra = nc.dram_tensor("ra", [2, cpad], F32R, kind="ExternalInput")
    rb = nc.dram_tensor("rb", [2, cpad], F32R, kind="ExternalInput")
    la = nc.dram_tensor("la", [2, rows], F32R, kind="ExternalInput")
    lb = nc.dram_tensor("lb", [2, rows], F32R, kind="ExternalInput")
    xs = nc.dram_tensor("xs", [P, BCOL], F32, kind="ExternalInput")
    ts = nc.dram_tensor("ts", [P, BCOL], F32, kind="ExternalInput")
    out = nc.dram_tensor("acc", [P, ncol], F32, kind="ExternalOutput")

    with tile.TileContext(nc) as tc, ExitStack() as ctx:
        pre = ctx.enter_context(tc.tile_pool(name="pre", bufs=1))
        uvp = ctx.enter_context(tc.tile_pool(name="uv", bufs=3))
        psp = ctx.enter_context(tc.tile_pool(name="psp", bufs=2, space="PSUM"))

        rat = pre.tile([2, cpad], F32R)
        rbt = pre.tile([2, cpad], F32R)
        lat = pre.tile([2, rows], F32R)
        lbt = pre.tile([2, rows], F32R)
        xst = pre.tile([P, BCOL], F32)
        tst = pre.tile([P, BCOL], F32)
        nc.sync.dma_start(out=rat, in_=ra.ap())
        nc.scalar.dma_start(out=rbt, in_=rb.ap())
        nc.sync.dma_start(out=lat, in_=la.ap())
        nc.scalar.dma_start(out=lbt, in_=lb.ap())
        nc.sync.dma_start(out=xst, in_=xs.ap())
        nc.scalar.dma_start(out=tst, in_=ts.ap())

        racc = pre.tile([P, ncol], F32)

        tix = 0
        for it in range(nb):
            lA = lat[:, it * P:(it + 1) * P]
            lB = lbt[:, it * P:(it + 1) * P]
            for jj in range(nb - it):
                jt = it + jj
                psA = psp.tile([P, JT], F32, tag="psA")
                psB = psp.tile([P, JT], F32, tag="psB")
                for h in range(JT // 512):
                    j0 = jt * JT + h * 512
                    nc.tensor.matmul(psA[:, h * 512:(h + 1) * 512],
                                     lhsT=lA, rhs=rat[:, j0:j0 + 512],
                                     start=True, stop=True)
                    nc.tensor.matmul(psB[:, h * 512:(h + 1) * 512],
                                     lhsT=lB, rhs=rbt[:, j0:j0 + 512],
                                     start=True, stop=True)
                U = uvp.tile([P, JT], F32, tag="U")
                nc.scalar.activation(U, psA, ACTF.Abs)
                # Signed product P = |D1| * D2 (only DVE can read PSUM), then
                # |P| = max(-P, P) in one stt per slice with fused row-sum
                # accum (Sxy partials). P is SBUF, so the idle gpsimd engine
                # absorbs the wide remainder of the abs pass.
                Pt = uvp.tile([P, JT], F32, tag="P")
                nc.vector.scalar_tensor_tensor(out=Pt, in0=U, scalar=0.0,
                                               in1=psB,
                                               op0=ALU.bypass, op1=ALU.mult)
                # |P| with fused row-sum accum, split ACT (activation Abs,
                # which idles otherwise) / DVE (stt max(-P, P)).
                Wt = uvp.tile([P, JT], F32, tag="W")
                nc.scalar.activation(Wt[:, 0:WD], Pt[:, 0:WD], ACTF.Abs,
                                     accum_out=racc[:, 2 * tix:2 * tix + 1])
                nc.vector.scalar_tensor_tensor(out=Wt[:, WD:JT],
                                               in0=Pt[:, WD:JT], scalar=-1.0,
                                               in1=Pt[:, WD:JT],
                                               op0=ALU.mult, op1=ALU.max,
                                               accum_out=racc[:, 2 * tix + 1:2 * tix + 2])
                tix += 1

        # BCE partials: softplus(x) = relu(x) + ln(1+exp(-|x|)), so
        # bce = (relu_sum + ln_sum - xt_sum)/N. Abs/Relu are in every act
        # table; Exp+Ln share natural_log_exp_and_others -> one reload max.
        rxj = pre.tile([P, BCOL], F32)
        nc.scalar.activation(rxj, xst, ACTF.Relu,
                             accum_out=racc[:, 2 * ntiles:2 * ntiles + 1])
        axj = pre.tile([P, BCOL], F32)
        nc.scalar.activation(axj, xst, ACTF.Abs)
        enj = pre.tile([P, BCOL], F32)
        nc.scalar.activation(enj, axj, ACTF.Exp, scale=-1.0)
        lnj = pre.tile([P, BCOL], F32)
        nc.scalar.activation(lnj, enj, ACTF.Ln, bias=1.0,
                             accum_out=racc[:, 2 * ntiles + 1:2 * ntiles + 2])
        xtj = pre.tile([P, BCOL], F32)
        nc.vector.scalar_tensor_tensor(out=xtj, in0=xst, scalar=0.0,
                                       in1=tst, op0=ALU.bypass, op1=ALU.mult,
                                       accum_out=racc[:, 2 * ntiles + 2:2 * ntiles + 3])


        nc.sync.dma_start(out=out.ap(), in_=racc)

    nc.compile()
    return nc


_NC_CACHE = {}


def _get_nc(cpad):
    if cpad not in _NC_CACHE:
        _NC_CACHE[cpad] = _build(cpad)
    return _NC_CACHE[cpad]


def _row_index(k, cpad):
    """Filtered-space row indices owned by core k (i-tiles NCORES*t + k)."""
    nit = cpad // JT
    idx = []
    for t in range(nit):
        g = NCORES * t + k
        idx.append(np.arange(g * P, (g + 1) * P))
    return np.concatenate(idx)


def _rowsums_abs(v):
    """A_i = sum_j |v_i - v_j| in O(c log c), exact float64."""
    o = np.argsort(v, kind="stable")
    q = v[o]
    pre = np.cumsum(q)
    tot = pre[-1]
    k = np.arange(len(q), dtype=np.float64)
    s = q * (2.0 * k + 2.0 - len(q)) - 2.0 * pre + tot
    a = np.empty_like(v)
    a[o] = s
    return a


def _numpy_loss(x, t, s):
    """Full-precision fallback (c > CPAD_BIG or degenerate inputs)."""
    x64 = x.astype(np.float64).reshape(-1)
    t64 = t.astype(np.float64).reshape(-1)
    s64 = s.astype(np.float64).reshape(-1)
    bce = np.mean(np.maximum(x64, 0) - x64 * t64 + np.log1p(np.exp(-np.abs(x64))))
    m = s.reshape(-1) >= np.float32(0.5)
    c = int(m.sum())
    if c == 0:
        return np.float32(bce)
    p = (1.0 / (1.0 + np.exp(-x64))).astype(np.float32)[m].astype(np.float64)
    ss = s.reshape(-1)[m].astype(np.float64)
    dx = np.abs(p[:, None] - p[None, :])
    dy = np.abs(ss[:, None] - ss[None, :])
    Sxy = float((dx * dy).sum())
    A = dx.sum(1); B = dy.sum(1)
    loss = bce + _assemble_dcor(c, Sxy, A, B, p, ss)
    return np.float32(loss)


def _assemble_dcor(c, Sxy, A, B, p, ss):
    Sxx = 2.0 * c * float((p * p).sum()) - 2.0 * float(p.sum()) ** 2
    Syy = 2.0 * c * float((ss * ss).sum()) - 2.0 * float(ss.sum()) ** 2
    Tx, Ty = float(A.sum()), float(B.sum())
    sAB, sAA, sBB = float(A @ B), float(A @ A), float(B @ B)
    Vxy = Sxy - 2.0 / c * sAB + Tx * Ty / c ** 2
    Vxx = Sxx - 2.0 / c * sAA + Tx * Tx / c ** 2
    Vyy = Syy - 2.0 / c * sBB + Ty * Ty / c ** 2
    EPS = 1e-8
    dcov = np.sqrt(max(Vxy / c ** 2, EPS))
    dvx = np.sqrt(max(Vxx / c ** 2, EPS))
    dvy = np.sqrt(max(Vyy / c ** 2, EPS))
    return dcov / (dvx * dvy)


def _prepare(inputs, targets, spectators):
    x = np.ascontiguousarray(np.asarray(inputs, dtype=np.float32)).reshape(N)
    t = np.ascontiguousarray(np.asarray(targets, dtype=np.float32)).reshape(N)
    s = np.ascontiguousarray(np.asarray(spectators, dtype=np.float32)).reshape(N)

    m = s >= np.float32(0.5)
    c = int(m.sum())
    cpad = CPAD if c <= CPAD else (CPAD_BIG if c <= CPAD_BIG else None)
    if cpad is None or c == 0:
        return None, None, None, (x, t, s)

    # p in f32, used consistently by host (A, Sxx) and device (Sxy)
    p32 = (1.0 / (1.0 + np.exp(-x.astype(np.float64)))).astype(np.float32)
    p_sel = p32[m]
    s_sel = s[m]

    p_pad = np.zeros(cpad, np.float32); p_pad[:c] = p_sel
    s_pad = np.zeros(cpad, np.float32); s_pad[:c] = s_sel
    m_pad = np.zeros(cpad, np.float32); m_pad[:c] = 1.0

    ra = np.ascontiguousarray(np.stack([m_pad, p_pad]))
    rb = np.ascontiguousarray(np.stack([m_pad, s_pad]))

    in_maps = []
    for k in range(NCORES):
        idx = _row_index(k, cpad)
        la = np.ascontiguousarray(np.stack([p_pad[idx], -m_pad[idx]]))
        lb = np.ascontiguousarray(np.stack([s_pad[idx], -m_pad[idx]]))
        xsk = np.ascontiguousarray(x[k * BSH:(k + 1) * BSH].reshape(P, BCOL))
        tsk = np.ascontiguousarray(t[k * BSH:(k + 1) * BSH].reshape(P, BCOL))
        in_maps.append({"ra": ra, "rb": rb, "la": la, "lb": lb,
                        "xs": xsk, "ts": tsk})

    meta = {
        "c": c, "cpad": cpad,
        "p_sel": p_sel.astype(np.float64),
        "s_sel": s_sel.astype(np.float64),
    }
    return cpad, in_maps, meta, None


def _combine(results, meta):
    cpad = meta["cpad"]
    nb = cpad // JT
    ntiles = nb * (nb + 1) // 2
    w = np.array(_tile_weights(nb), np.float64)

    Sxy = 0.0
    sp_sum = 0.0
    xt_sum = 0.0
    for res in results:
        cols = res["acc"].astype(np.float64).sum(axis=0)
        rt = cols[:2 * ntiles].reshape(ntiles, 2).sum(axis=1)
        Sxy += float(rt @ w)
        sp_sum += float(cols[2 * ntiles]) + float(cols[2 * ntiles + 1])
        xt_sum += float(cols[2 * ntiles + 2])

    bce = (sp_sum - xt_sum) / N
    c = meta["c"]
    p = meta["p_sel"]; ss = meta["s_sel"]
    A = _rowsums_abs(p); B = _rowsums_abs(ss)
    loss = bce + _assemble_dcor(c, Sxy, A, B, p, ss)
    return np.float32(loss)


def kernel(inputs, targets, spectators):
    cpad, in_maps, meta, fb = _prepare(inputs, targets, spectators)
    if fb is not None:
        return _numpy_loss(*fb)
    nc = _get_nc(cpad)
    res = run_bass_kernel_spmd(nc, in_maps, list(range(NCORES)))
    return _combine(res.results, meta)


if __name__ == "__main__":
    d = np.load("/root/problem/cached_io.npz")
    out = kernel(d["inputs"], d["targets"], d["spectators"])
    exp = float(d["expected"])
    rel = abs(float(out) - exp) / abs(exp)
    print(f"kernel: {float(out):.8f}  expected: {exp:.8f}  rel err: {rel:.3e}")
